# revision 104
# baseline (speedup 1.0000x reference)
"""MFGCGRU (graph-conv GRU cell) Trainium2 kernel — fp8 DoubleRow edition.

Strategy: data-parallel over batch B=32 across 8 NeuronCores (4 batches
per core). The three N-contraction supports (adj1, adj2, exp-attention)
run as fp8e4 DoubleRow matmuls (K=256 per MM, 2 fp8 mults/cell/cycle),
which doubles Tensor-engine throughput over bf16. Scales keep fp8
operands in the normal range: adjacency x8, Y = X@k x4 (folded into the
kernels host-side); the identity kernels carry x32 and the final
activation scale divides it back out.

Layouts (per core):
  - x^T [66, N] bf16, rows 0:64 = h, 64:66 = inputs (kernel rows
    permuted to match); h rows are overwritten with r*h in phase 1.
  - adjacency in DR pair layout [128, g, i, w] fp8: node n = g*256 +
    i*128 + p is (partition p, pair slot i) of 256-group g.
  - Y tiles [128, g, i, m, u] fp8 (m in {a1, a2, att}); stationary
    slice [:, g, :, m, :] is the DR pair operand.
  - e^T = exp(K Q^T / 8) is generated once per tile into a persistent
    fp8 [128, g, i, w] buffer (pipelined one tile ahead of its
    consumers) and reused by phase 2; its row-normalizer 1/d is
    partition-broadcast into rdbc[t] and applied to the e-contraction
    PSUM (the attention kernel slot carries the full xSC scale).
  - sigmoid is computed as 0.5*tanh(z/2)+0.5 so the ACT engine never
    switches activation tables between exp and sigmoid.
"""

import contextlib
import os

import numpy as np
import ml_dtypes

import concourse.bass as bass
import concourse.bacc as bacc
import concourse.tile as tile
from concourse import mybir
from concourse.bass_utils import run_bass_kernel_spmd

F32 = mybir.dt.float32
BF16 = mybir.dt.bfloat16
FP8 = mybir.dt.float8e4
AF = mybir.ActivationFunctionType
ALU = mybir.AluOpType
DR = mybir.MatmulPerfMode.DoubleRow

B, N, DIN, U, FD, SD = 32, 2048, 2, 64, 32, 64
NCORES = 8
BL = B // NCORES          # batches per core
NTW = 512                 # n-tile width
NT = N // NTW             # 4 n-tiles
NBW = 128                 # node-block width
NB = N // NBW             # 16 node blocks
NG = NB // 2              # 8 DoubleRow 256-groups
FROWS = DIN + U           # 66
SA = 8.0                  # adjacency fp8 scale
SY = 4.0                  # Y fp8 scale
SC = SA * SY              # combined scale on the identity path


def _build_program():
    nc = bacc.Bacc("TRN2", debug=False, num_devices=NCORES)

    d = {}

    def din(name, shape, dt):
        d[name] = nc.dram_tensor(name, shape, dt, kind="ExternalInput").ap()

    din("xT", [BL, FROWS, N], BF16)
    din("hT", [BL, U, N], F32)
    din("a1p", [128, NG, 2, N], FP8)
    din("a2p", [128, NG, 2, N], FP8)
    din("fsTx", [FD + SD, N + 2 * 2 * FD], BF16)
    din("xt0k", [FROWS, N + 3 * 2 * U], BF16)
    din("ws1", [FD + SD, U], BF16)
    din("bsp", [2 * U, 3], F32)
    din("ws2", [U, 1], BF16)
    din("kk0", [FROWS, 2 * U], BF16)
    din("kcall", [FROWS, 3 * U], BF16)
    din("kc0", [FROWS, U], BF16)
    din("bc2", [2 * U, 1], F32)
    out_h = nc.dram_tensor("out", [BL, U, N], F32, kind="ExternalOutput").ap()

    with tile.TileContext(nc) as tc:
        _emit(tc, d, out_h)
    nc.compile()
    return nc


def _emit(tc, d, out_h):
    nc = tc.nc
    ctx = contextlib.ExitStack()
    const = ctx.enter_context(tc.tile_pool(name="const", bufs=1))
    persist = ctx.enter_context(tc.tile_pool(name="persist", bufs=1))
    adjp = ctx.enter_context(tc.tile_pool(name="adjp", bufs=8))
    stage = ctx.enter_context(tc.tile_pool(name="stage", bufs=2))
    stage1 = ctx.enter_context(tc.tile_pool(name="stage1", bufs=1))
    psacc = ctx.enter_context(tc.tile_pool(name="psacc", bufs=6, space="PSUM"))
    pse = ctx.enter_context(tc.tile_pool(name="pse", bufs=1, space="PSUM"))
    psmall = psacc

    # ---- constants / weights in SBUF ----
    def cload(name):
        ap = d[name]
        t = const.tile(list(ap.shape), ap.dtype, name=f"c_{name}")
        nc.sync.dma_start(out=t, in_=ap)
        return t

    # DMA issue order doubles as priority on the (serialized) DMA device:
    # everything the upfront PE segment reads goes before the big adjacency
    # transfers; phase-2-only constants go last.  Small constants are packed
    # into the first big transfers so the PE's first matmuls wait on as few
    # DMA round-trips as possible.
    fsTx = const.tile([FD + SD, N + 4 * FD], BF16, name="c_fsTx")
    nc.sync.dma_start(out=fsTx[0:FD, :], in_=d["fsTx"][0:FD, :])
    fsT = fsTx[:, 0:N]
    wqk = fsTx[0:FD, N:N + 4 * FD]

    # ---- persistent activations (xT0 carries kkall in its tail cols) ----
    xt0k = persist.tile([FROWS, N + 3 * 2 * U], BF16, name="xt0k", tag="xT0")
    nc.sync.dma_start(out=xt0k, in_=d["xt0k"])
    kkall = xt0k[:, N:]
    xT = [xt0k[:, 0:N]] + [persist.tile([FROWS, N], BF16, name=f"xT{b}",
                                        tag=f"xT{b}") for b in range(1, BL)]
    nc.sync.dma_start(out=fsTx[FD:, :], in_=d["fsTx"][FD:, :])
    ws1 = cload("ws1")
    for b in range(1, BL):
        nc.sync.dma_start(out=xT[b], in_=d["xT"][b])
    bsp = cload("bsp")                       # [128, 3] f32 bias pack
    bruh = bsp[:, 0:1]
    bs1v = bsp[0:U, 1:2]
    bs2v = bsp[0:1, 2:3]
    ws2 = cload("ws2")
    kk0 = cload("kk0")

    # phase-1 adjacency slices for tiles 0 and 1 (the rest prefetch in-loop)
    def adjslice(name, t):
        a = adjp.tile([128, NG, 2, NTW], FP8, name=f"sl_{name}_{t}", tag="adj")
        nc.sync.dma_start(out=a, in_=d[name][:, :, :, t * NTW:(t + 1) * NTW])
        return a

    adjq = {}
    for t in (0, 1):
        adjq[(1, t)] = adjslice("a1p", t)
        adjq[(2, t)] = adjslice("a2p", t)

    kcall = cload("kcall")
    kc0 = cload("kc0")
    bc2 = cload("bc2")

    # DoubleRow weights need a pair-step that is a multiple of 16 bytes
    ones2w = const.tile([128, 2, 16], FP8, name="ones2w")
    nc.vector.memset(ones2w, 1.0)
    ones2 = ones2w[:, :, 0:1]

    # warm-up: keep the PE busy (and ramping) while the first input DMAs
    # are in flight; results are never read
    warm = psacc.tile([16, 32], F32, name="warm", tag="acc")
    for _ in range(96):
        nc.tensor.matmul(warm, ones2w[:, 0, 0:16],
                         ones2w.rearrange("p i x -> p (i x)"),
                         start=True, stop=True)

    QTp = persist.tile([FD, 2, N], FP8, name="QTp", tag="QTp")
    KTp = persist.tile([FD, 2, N], FP8, name="KTp", tag="KTp")
    s_row = persist.tile([1, N], BF16, name="s_row", tag="s_row")
    rdbc = [persist.tile([128, NTW], BF16, name=f"rdbc{t}", tag=f"rdbc{t}")
            for t in range(NT)]
    et = [persist.tile([128, NG, 2, NTW], FP8, name=f"et{t}", tag=f"et{t}")
          for t in range(NT)]
    y = [persist.tile([128, NG, 2, 3, NBW], FP8, name=f"y_{b}", tag=f"y{b}")
         for b in range(BL)]
    yc = [persist.tile([128, NG, 2, 3, 2 * U], FP8, name=f"yc_{p}",
                       tag=f"yc{p}") for p in range(BL // 2)]
    usb = [persist.tile([128, N], BF16, name=f"usb{p}", tag=f"usb{p}")
           for p in range(BL // 2)]
    # pair-packed copy of h (bf16), stashed before phase 1 overwrites the
    # h rows of xT with r*h — saves the per-tile hp DMAs in phase 2
    hsb = [persist.tile([128, N], BF16, name=f"hsb{p}", tag=f"hsb{p}")
           for p in range(BL // 2)]
    for p in range(BL // 2):
        for half in range(2):
            nc.sync.dma_start(out=hsb[p][half * U:(half + 1) * U, :],
                              in_=xT[2 * p + half][0:U, :])

    # round-robin engine picker for PSUM->SBUF evacuation copies
    # (GPSIMD cannot access PSUM on this target, so DVE/ACT only)
    cp_engines = [nc.vector, nc.scalar]
    cp_idx = [0]

    def copy_out(dst, src):
        eng = cp_engines[cp_idx[0] % len(cp_engines)]
        cp_idx[0] += 1
        if eng is nc.scalar:
            nc.scalar.activation(dst, src, AF.Copy)
        else:
            eng.tensor_copy(dst, src)

    def interleave(main, extra, ratio=2):
        mi = ei = 0
        while mi < len(main) or ei < len(extra):
            for _ in range(ratio):
                if mi < len(main):
                    main[mi](); mi += 1
            if ei < len(extra):
                extra[ei](); ei += 1

    # ---- prelude: Q^T, K^T (fp8 pair layout) and sentinel s.
    # Batched so the PE never sits in a MM->DVE->MM serial chain: all QK
    # matmuls, then the s1 matmuls, then (after the DVE bias+relu drain)
    # the tiny s2 matmuls.
    def prelude_qk_thunks():
        th = []
        for t in range(NT):
            def f(t=t):
                _emit_qk_tile(t)
            th.append(f)
        return th

    def _emit_qk_tile(t):
        if True:
            sl = slice(t * NTW, (t + 1) * NTW)
            # one MM computes Q and K for both pair slots; the relu
            # evacuations move partition blocks 32i -> base 0 (cross-base)
            pqk = psmall.tile([128, NTW], F32, name="pqk", tag="acc")
            nc.tensor.matmul(pqk, wqk, fsT[0:FD, sl], start=True, stop=True)
            for j, (dstP, i) in enumerate(
                    ((QTp, 0), (QTp, 1), (KTp, 0), (KTp, 1))):
                src = pqk[FD * j:FD * (j + 1), :]
                if j % 2 == 0:
                    nc.scalar.activation(dstP[:, i, sl], src, AF.Relu)
                else:
                    nc.vector.tensor_scalar(
                        out=dstP[:, i, sl], in0=src, scalar1=0.0,
                        scalar2=None, op0=ALU.max)

    def s_thunks():
        """The sentinel-score path, interleaved into y-gen later on."""
        th = []
        s1ts = {}
        for t in range(NT):
            def f1(t=t):
                sl = slice(t * NTW, (t + 1) * NTW)
                ps1 = psmall.tile([U, NTW], F32, name="ps1", tag="acc")
                nc.tensor.matmul(ps1, ws1, fsT[:, sl], start=True, stop=True)
                s1t = stage1.tile([U, NTW], BF16, name=f"s1t{t}",
                                  tag=f"s1t{t % 2}")
                nc.vector.tensor_scalar(out=s1t, in0=ps1, scalar1=bs1v,
                                        scalar2=0.0, op0=ALU.add, op1=ALU.max)
                s1ts[t] = s1t
            th.append(f1)
        for t in range(NT):
            def f2(t=t):
                sl = slice(t * NTW, (t + 1) * NTW)
                ps2 = psmall.tile([1, NTW], F32, name="ps2", tag="acc")
                nc.tensor.matmul(ps2, ws2, s1ts[t], start=True, stop=True)
                nc.vector.tensor_scalar(out=s_row[:, sl], in0=ps2,
                                        scalar1=bs2v, scalar2=0.0,
                                        op0=ALU.add, op1=ALU.max)
            th.append(f2)
        return th

    # ---- y generation (phase 1): Y[b] = X_b @ [kr|ku][m], fp8 x4 ----
    def ygen_thunks(b):
        th = []
        for jb in range(NB):
            def f(b=b, jb=jb):
                nsl = slice(jb * NBW, (jb + 1) * NBW)
                py = psacc.tile([128, 3 * 2 * U], F32, name="py", tag="acc")
                nc.tensor.matmul(py, xT[b][:, nsl], kkall, start=True,
                                 stop=True)
                g, i = divmod(jb, 2)
                copy_out(y[b][:, g, i, :, :],
                         py.rearrange("p (m u) -> p m u", m=3))
            th.append(f)
        return th

    # ---- e^T generation for tile t (into persistent et[t]) ----
    def e_thunks(t):
        sl = slice(t * NTW, (t + 1) * NTW)
        th = []
        for g in range(NG):
            def f(g=g, t=t, sl=sl):
                pe2 = pse.tile([128, 2 * NTW], F32, name="pe2", tag="pse")
                for i in range(2):
                    nsl = slice((2 * g + i) * NBW, (2 * g + i + 1) * NBW)
                    nc.tensor.matmul(pe2[:, i * NTW:(i + 1) * NTW],
                                     KTp[:, :, nsl], QTp[:, :, sl],
                                     start=True, stop=True, perf_mode=DR)
                nc.scalar.activation(et[t][:, g, :, :], pe2, AF.Exp,
                                     scale=0.125)
            th.append(f)
        return th

    # ---- d colsum + rdbc[t] = SA / d  (consumes complete et[t]) ----
    def d_rdbc_thunks(t):
        sl = slice(t * NTW, (t + 1) * NTW)
        th = []
        pd_box = [None]

        def mk(g):
            def f(g=g):
                if g == 0:
                    pd_box[0] = psmall.tile([1, NTW], F32, name="pd",
                                            tag="acc")
                nc.tensor.matmul(pd_box[0], ones2, et[t][:, g, :, :],
                                 start=(g == 0), stop=(g == NG - 1),
                                 perf_mode=DR)
            return f
        th += [mk(g) for g in range(NG)]

        def tail(t=t, sl=sl):
            dsb = stage.tile([1, NTW], F32, name="dsb", tag="dsb")
            nc.vector.tensor_add(dsb, pd_box[0], s_row[:, sl])
            rds = stage.tile([1, NTW], BF16, name="rds", tag="dsb")
            with nc.allow_low_precision(reason="1/d broadcast factor"):
                nc.vector.reciprocal(rds, dsb)
            nc.gpsimd.partition_broadcast(rdbc[t], rds)
        th.append(tail)
        return th

    # =================== phase 1: r & u gates ===================
    def a_thunks1(b, t, a1, a2, pa_box):
        sl = slice(t * NTW, (t + 1) * NTW)

        def first():
            pa_box[0] = psacc.tile([128, NTW], F32, name=f"pa{b}", tag="acc")
            nc.tensor.matmul(pa_box[0], kk0, xT[b][:, sl],
                             start=True, stop=False)
        th = [first]
        for m, asl in ((0, a1), (1, a2)):
            for g in range(NG):
                def f(m=m, asl=asl, g=g):
                    nc.tensor.matmul(pa_box[0], y[b][:, g, :, m, :],
                                     asl[:, g, :, :], start=False,
                                     stop=(m == 1 and g == NG - 1),
                                     perf_mode=DR)
                th.append(f)
        return th

    def b_thunks1(b, t, pa_box):
        """8 e-contraction MMs plus the non-PE gate tail, as thunks."""
        sl = slice(t * NTW, (t + 1) * NTW)
        pb_box = [None]

        def mk(g):
            def f(g=g):
                if g == 0:
                    pb_box[0] = psacc.tile([128, NTW], F32, name="pb",
                                           tag="acc")
                nc.tensor.matmul(pb_box[0], y[b][:, g, :, 2, :],
                                 et[t][:, g, :, :], start=(g == 0),
                                 stop=(g == NG - 1), perf_mode=DR)
            return f
        th = [mk(g) for g in range(NG)]

        def tail():
            pa, pb = pa_box[0], pb_box[0]
            tmp = stage.tile([128, NTW], F32, name="tmp", tag="tmp")
            nc.vector.tensor_mul(tmp, pb, rdbc[t])
            nc.vector.tensor_add(pa, pa, tmp)
            tht = stage.tile([128, NTW], F32, name="tht", tag="sig")
            nc.scalar.activation(tht, pa, AF.Tanh, scale=0.125 / SC,
                                 bias=bruh)
            sig = stage.tile([128, NTW], BF16, name="sig", tag="sig2")
            nc.gpsimd.tensor_scalar(out=sig, in0=tht, scalar1=0.5,
                                    scalar2=0.5, op0=ALU.mult, op1=ALU.add)
            nc.gpsimd.tensor_mul(xT[b][0:U, sl], sig[0:U, :], xT[b][0:U, sl])
            p, half = divmod(b, 2)
            nc.sync.dma_start(out=usb[p][half * U:(half + 1) * U, sl],
                              in_=sig[U:128, :])
        th.append(tail)
        return th

    # ---- yc generation (phase 2 inputs; xT rows 0:64 hold r*h) ----
    # jbs selects node blocks: block jb reads xT tile jb//4, whose r*h
    # update must already be EMITTED (emission order is semantics).
    def ycgen_thunks(p, jbs=None):
        th = []
        for half in range(2):
            b = 2 * p + half
            usl = slice(half * U, (half + 1) * U)
            for jb in (range(NB) if jbs is None else jbs):
                def f(b=b, usl=usl, jb=jb, p=p):
                    nsl = slice(jb * NBW, (jb + 1) * NBW)
                    pyc = psacc.tile([128, 3 * U], F32, name="pyc",
                                     tag="acc")
                    nc.tensor.matmul(pyc, xT[b][:, nsl], kcall,
                                     start=True, stop=True)
                    g, i = divmod(jb, 2)
                    copy_out(yc[p][:, g, i, :, usl],
                             pyc.rearrange("p (m u) -> p m u", m=3))
                th.append(f)
        return th

    # ---------- upfront segment ----------
    # only y[0] / y[1] are generated before the tile loop; y[2] / y[3]
    # ride inside tile 0, which has engine slack for their evacuations
    for f in prelude_qk_thunks():
        f()
    interleave(ygen_thunks(0) + ygen_thunks(1), s_thunks() + e_thunks(0),
               ratio=4)
    for f in d_rdbc_thunks(0):
        f()

    # ---------- phase-1 tile loop ----------
    for t in range(NT):
        sl = slice(t * NTW, (t + 1) * NTW)
        if t >= 1 and t + 1 < NT:
            adjq[(1, t + 1)] = adjslice("a1p", t + 1)
            adjq[(2, t + 1)] = adjslice("a2p", t + 1)
        a1 = adjq[(1, t)]
        a2 = adjq[(2, t)]

        pab = [[None] for _ in range(BL)]

        def ab(b):
            return (a_thunks1(b, t, a1, a2, pab[b])
                    + b_thunks1(b, t, pab[b]))

        if t == 0:
            # tile 0 absorbs the y[2]/y[3] generation in its slack
            interleave(ab(0) + ab(1), ygen_thunks(2) + e_thunks(1)[:3],
                       ratio=3)
            interleave(ab(2), ygen_thunks(3), ratio=2)
            interleave(ab(3), e_thunks(1)[3:], ratio=5)
        else:
            # next-tile e-gen spreads across the FULL tile so the
            # single-buffered psum<->exp chain never backs up into PE
            extras = (e_thunks(t + 1) if t + 1 < NT
                      else ycgen_thunks(0, range(3 * NB // 4)))
            main = ab(0) + ab(1) + ab(2) + ab(3)
            ratio = max(1, len(main) // max(1, len(extras)))
            interleave(main, extras, ratio=ratio)
        if t + 1 < NT:
            for f in d_rdbc_thunks(t + 1):
                f()

    # =================== phase 2: c gate & h_new ===================
    # (adjacency tiles stay resident from phase 1 — no re-DMA)
    for f in (ycgen_thunks(0, range(3 * NB // 4, NB))
              + ycgen_thunks(1)):
        f()

    # h_new = u*h + (1-u)*c: uh and om := 1-u depend only on usb/hsb, so
    # they are prepared one tile AHEAD, fully off the ct critical path;
    # their tags reuse phase-1-only rings (dsb / sig2) to save SBUF.
    def uhom_prep(t):
        sl = slice(t * NTW, (t + 1) * NTW)
        uhs, oms = [], []
        for p in range(BL // 2):
            uh = stage.tile([128, NTW], F32, name=f"uh{p}_{t}", tag="dsb")
            nc.vector.tensor_mul(uh, usb[p][:, sl], hsb[p][:, sl])
            om = stage.tile([128, NTW], BF16, name=f"om{p}_{t}", tag="sig2")
            nc.gpsimd.tensor_scalar(out=om, in0=usb[p][:, sl], scalar1=-1.0,
                                    scalar2=1.0, op0=ALU.mult, op1=ALU.add)
            uhs.append(uh)
            oms.append(om)
        return uhs, oms

    prep = uhom_prep(0)

    def emit_p2(t, a1, a2, uhs, oms, c0, cw):
        gsl = slice(t * NTW + c0, t * NTW + c0 + cw)
        ccs = slice(c0, c0 + cw)
        for p in range(BL // 2):
            b0, b1 = 2 * p, 2 * p + 1
            pa = psacc.tile([128, cw], F32, name="pa2c", tag="acc")
            nc.tensor.matmul(pa, yc[p][:, 0, :, 0, :], a1[:, 0, :, ccs],
                             start=True, stop=False, perf_mode=DR)
            nc.tensor.matmul(pa[0:U, :], kc0, xT[b0][:, gsl],
                             start=False, stop=False)
            nc.tensor.matmul(pa[U:128, :], kc0, xT[b1][:, gsl],
                             start=False, stop=False)
            for m, asl in ((0, a1), (1, a2)):
                for g in range(NG):
                    if m == 0 and g == 0:
                        continue
                    nc.tensor.matmul(pa, yc[p][:, g, :, m, :],
                                     asl[:, g, :, ccs], start=False,
                                     stop=(m == 1 and g == NG - 1),
                                     perf_mode=DR)
            pb = psacc.tile([128, cw], F32, name="pb2", tag="acc")
            for g in range(NG):
                nc.tensor.matmul(pb, yc[p][:, g, :, 2, :],
                                 et[t][:, g, :, ccs], start=(g == 0),
                                 stop=(g == NG - 1), perf_mode=DR)
            tmp = stage.tile([128, cw], F32, name="tmp2", tag="tmp")
            ct = stage.tile([128, cw], F32, name="ct", tag="sig")
            t1 = stage.tile([128, cw], F32, name="t1", tag="t1")
            # halves pipeline DVE/ACT/Pool mid-stream; the final tile runs
            # full-width so its drain chain is as short as possible
            hw_ = max(cw // 2, NTW // 2)
            for d0 in range(0, cw, hw_):
                cs = slice(d0, min(d0 + hw_, cw))
                ocs = slice(c0 + d0, c0 + min(d0 + hw_, cw))
                nc.vector.tensor_mul(tmp[:, cs], pb[:, cs], rdbc[t][:, ocs])
                nc.vector.tensor_add(pa[:, cs], pa[:, cs], tmp[:, cs])
                nc.scalar.activation(ct[:, cs], pa[:, cs], AF.Tanh,
                                     scale=0.25 / SC, bias=bc2)
                # last tile: keep the drain chain on DVE (shorter hops)
                omeng = nc.vector if t == NT - 1 else nc.gpsimd
                omeng.tensor_mul(ct[:, cs], oms[p][:, ocs], ct[:, cs])
                nc.vector.tensor_add(t1[:, cs], uhs[p][:, ocs], ct[:, cs])
            # issue the store from the ACT queue: on SP it would head-of-line
            # block other prefetches until t1 is ready
            if t == NT - 1:
                h2 = cw // 2
                for d0 in (0, h2):
                    osl = slice(t * NTW + c0 + d0, t * NTW + c0 + d0 + h2)
                    nc.scalar.dma_start(out=out_h[2 * p:2 * p + 2, :, osl],
                                        in_=t1[:, d0:d0 + h2])
            else:
                nc.scalar.dma_start(out=out_h[2 * p:2 * p + 2, :, gsl],
                                    in_=t1)

    for t in range(NT):
        a1 = adjq[(1, t)]
        a2 = adjq[(2, t)]
        uhs, oms = prep
        if t + 1 < NT:
            prep = uhom_prep(t + 1)
        emit_p2(t, a1, a2, uhs, oms, 0, NTW)

    ctx.close()


_CACHE = {}


def _get_program():
    if "nc" not in _CACHE:
        _CACHE["nc"] = _build_program()
    return _CACHE["nc"]


def _prep_inputs(inputs, h_prev, adj1, adj2, feat, SE, Wq, Wk, Ws1, bs1, Ws2,
                 bs2, r_kernel, r_bias, u_kernel, u_bias, c_kernel, c_bias):
    bf = ml_dtypes.bfloat16
    f8 = ml_dtypes.float8_e4m3
    f32 = np.float32
    perm = list(range(DIN, FROWS)) + list(range(DIN))  # [h(64); inputs(2)]

    h3 = np.asarray(h_prev, f32).reshape(B, N, U)
    hT = np.ascontiguousarray(h3.transpose(0, 2, 1))            # [B, U, N]
    inT = np.asarray(inputs, f32).transpose(0, 2, 1)            # [B, DIN, N]
    xT = np.concatenate([hT, inT], axis=1).astype(bf)           # [B, 66, N]

    rk = np.asarray(r_kernel, f32)[:, perm, :]
    uk = np.asarray(u_kernel, f32)[:, perm, :]
    ck = np.asarray(c_kernel, f32)[:, perm, :]
    # a-support slots carry xSY (their adjacency carries xSA); the attention
    # slot carries the full xSC since its normalizer rdbc is a plain 1/d
    msc = {1: SY, 2: SY, 3: SC}
    kkall = np.concatenate(
        [np.concatenate([rk[m], uk[m]], axis=1) * msc[m] for m in (1, 2, 3)],
        axis=1).astype(bf)                                      # [66, 384]
    kk0 = (np.concatenate([rk[0], uk[0]], axis=1) * SC).astype(bf)
    kcall = np.concatenate(
        [ck[m] * msc[m] for m in (1, 2, 3)], axis=1).astype(bf)
    kc0 = (ck[0] * SC).astype(bf)

    def adjpack(a):
        at = np.ascontiguousarray(np.asarray(a, f32).T) * SA    # [n, w]
        return np.ascontiguousarray(
            at.reshape(NG, 2, 128, N).transpose(2, 0, 1, 3)).astype(f8)

    wqk = np.concatenate(
        [np.asarray(Wq, f32)[:, 0:FD], np.asarray(Wq, f32)[:, FD:U],
         np.asarray(Wk, f32)[:, 0:FD], np.asarray(Wk, f32)[:, FD:U]],
        axis=1)
    fsTx = np.zeros((FD + SD, N + 4 * FD), f32)
    fsTx[0:FD, 0:N] = np.asarray(feat, f32).T
    fsTx[FD:, 0:N] = np.asarray(SE, f32).T
    fsTx[0:FD, N:] = wqk
    bsp = np.zeros((2 * U, 3), f32)
    bsp[:, 0] = 0.5 * np.concatenate(
        [np.asarray(r_bias, f32).mean(0), np.asarray(u_bias, f32).mean(0)])
    bsp[0:U, 1] = np.asarray(bs1, f32)
    bsp[0, 2] = np.asarray(bs2, f32).reshape(())

    shared = {
        "a1p": adjpack(adj1),
        "a2p": adjpack(adj2),
        "fsTx": np.ascontiguousarray(fsTx).astype(bf),
        "ws1": np.asarray(Ws1, f32).astype(bf),
        "bsp": bsp,
        "ws2": np.asarray(Ws2, f32).reshape(U, 1).astype(bf),
        "kk0": kk0,
        "kcall": kcall,
        "kc0": kc0,
        "bc2": np.tile(np.asarray(c_bias, f32).mean(0), 2).reshape(-1, 1),
    }
    in_maps = []
    for c in range(NCORES):
        bsl = slice(c * BL, (c + 1) * BL)
        m = dict(shared)
        xTc = np.ascontiguousarray(xT[bsl])
        m["xT"] = xTc
        m["xt0k"] = np.ascontiguousarray(
            np.concatenate([xTc[0], kkall], axis=1))
        m["hT"] = np.ascontiguousarray(hT[bsl])
        in_maps.append(m)
    return in_maps


def kernel(**inputs):
    os.environ.setdefault("NEURON_RT_RESET_CORES", "1")
    nc = _get_program()
    in_maps = _prep_inputs(**inputs)
    res = None
    err = None
    for _ in range(2):
        try:
            res = run_bass_kernel_spmd(nc, in_maps, list(range(NCORES)))
            break
        except Exception as e:  # e.g. a wedged device; retry once
            err = e
    if res is None:
        raise err
    outs = []
    for c in range(NCORES):
        o = res.results[c]["out"]                     # [BL, U, N] f32
        outs.append(o.transpose(0, 2, 1).reshape(BL, N * U))
    return np.concatenate(outs, axis=0).astype(np.float32)
